# revision 128
# baseline (speedup 1.0000x reference)
"""Bass/Tile TRN2 kernel for nn_LocalTransformerBlock — v2 (software-pipelined).

Data-parallel: one batch element per core. Two passes over the 32 row-tiles:
pass A (attention) keeps x2 resident in SBUF, pass B (FFN) reads it back.
Matmuls bf16; PSUM->SBUF copies bank-packed; rsqrt on DVE (quake + Newton);
scalar engine uses only {Copy, Exp} in pass A and {Copy, Gelu} in pass B
(2 act-table loads total).

LN trick: q/k l2norm cancels LN1's rstd, so QKV runs on centered-only h;
rstd1 scales V (DVE tensor_scalar during the V copy) and 8/|q| scales q
rows before the qT transpose.  Causal mask is multiplicative on A (Pool,
post-exp); row sums via DVE f16 reduce.
"""
import numpy as np
from contextlib import ExitStack

import concourse.bass as bass
import concourse.bacc as bacc
import concourse.tile as tile
from concourse import masks as cmasks
from concourse import mybir
from concourse.bass_utils import run_bass_kernel_spmd

DIM = 512
HEADS = 8
DHEAD = 64
WIN = 128
NTOK = 4096
NT = NTOK // WIN
B = 8
QK_SCALE = 8.0

F32 = mybir.dt.float32
BF16 = mybir.dt.bfloat16
F16 = mybir.dt.float16
I32 = mybir.dt.int32
FP8 = mybir.dt.float8e4
DR = mybir.MatmulPerfMode.DoubleRow
W8SCALE = 8.0
AF = mybir.ActivationFunctionType
ALU = mybir.AluOpType
RING = 6


def _bc(ap, dims):
    return bass.AP(tensor=ap.tensor, offset=ap.offset, ap=dims)


def build_program():
    nc = bacc.Bacc()

    x_d = nc.declare_dram_parameter("x", [NTOK, DIM], F32, isOutput=False)
    wqkvT_d = nc.declare_dram_parameter("wqkvT", [DIM, 3 * DIM], FP8, isOutput=False)
    woutT_d = nc.declare_dram_parameter("woutT", [DIM, DIM], FP8, isOutput=False)
    wff1T_d = nc.declare_dram_parameter("wff1T", [DIM, 4 * DIM], FP8, isOutput=False)
    wff2T_d = nc.declare_dram_parameter("wff2T", [4 * DIM, DIM], FP8, isOutput=False)
    rope_d = nc.declare_dram_parameter("rope", [NTOK, 4 * DHEAD], BF16, isOutput=False)
    mask_d = nc.declare_dram_parameter("mask", [WIN, WIN], BF16, isOutput=False)
    out_d = nc.declare_dram_parameter("out", [NTOK, DIM], F32, isOutput=True)

    with ExitStack() as ctx:
        tc = ctx.enter_context(tile.TileContext(nc))
        consts = ctx.enter_context(tc.tile_pool(name="consts", bufs=1))
        io = ctx.enter_context(tc.tile_pool(name="io", bufs=7))
        work = ctx.enter_context(tc.tile_pool(name="work", bufs=4))
        hpool = ctx.enter_context(tc.tile_pool(name="hpool", bufs=6))
        small = ctx.enter_context(tc.tile_pool(name="small", bufs=8))
        apool = ctx.enter_context(tc.tile_pool(name="apool", bufs=5))
        psT = ctx.enter_context(tc.tile_pool(name="psT", bufs=3, space="PSUM"))
        psQ = ctx.enter_context(tc.tile_pool(name="psQ", bufs=2, space="PSUM"))
        psS = ctx.enter_context(tc.tile_pool(name="psS", bufs=2, space="PSUM"))
        psY = ctx.enter_context(tc.tile_pool(name="psY", bufs=1, space="PSUM"))

        # ---- resident weights ----
        wq_sb = consts.tile([128, 4, 3 * DIM], FP8)
        wo_sb = consts.tile([128, 4, DIM], FP8)
        wf1_sb = consts.tile([128, 4, 4 * DIM], FP8)
        wf2_sb = consts.tile([128, 16, DIM], FP8)
        for k in range(4):
            nc.sync.dma_start(out=wq_sb[:, k, :], in_=wqkvT_d[k * 128:(k + 1) * 128, :])
            nc.sync.dma_start(out=wo_sb[:, k, :], in_=woutT_d[k * 128:(k + 1) * 128, :])

        def load_ffn_weights(j):
            # one FFN-weight DMA per early A-iteration (needed only in pass B)
            if j < 4:
                nc.sync.dma_start(out=wf1_sb[:, j, :],
                                  in_=wff1T_d[j * 128:(j + 1) * 128, :])
            elif j < 20:
                k = j - 4
                nc.sync.dma_start(out=wf2_sb[:, k, :],
                                  in_=wff2T_d[k * 128:(k + 1) * 128, :])

        eye_bf = consts.tile([128, 128], BF16)
        cmasks.make_identity(nc, eye_bf[:, :])
        eyeW8 = consts.tile([128, 128], BF16)
        cmasks.make_identity(nc, eyeW8[:, :])
        nc.gpsimd.tensor_scalar(out=eyeW8[:, :], in0=eyeW8[:, :],
                                scalar1=W8SCALE, scalar2=None, op0=ALU.mult)
        eye_f16 = consts.tile([128, 128], F16)
        cmasks.make_identity(nc, eye_f16[:, :])
        mask_sb = consts.tile([128, WIN], BF16)
        nc.sync.dma_start(out=mask_sb, in_=mask_d[:, :])

        x2_slab = consts.tile([128, NT, DIM], BF16)

        # rings: per-tile kT / V slabs (depth RING)
        # kT layout: partition = (h%2)*64 + d, dim2 = h//2 (qT4-matching order)
        kslab = consts.tile([128, RING, 4, WIN], BF16)
        vslab = consts.tile([128, RING, HEADS * DHEAD], F16)

        def pair_bc(dup, n, reps):
            """[128, n, 2] duplicated-pair tile -> broadcast AP
            [128, n, reps//2, 2] with packed last dim (keeps DVE 2x_1p)."""
            return bass.AP(tensor=dup.tensor, offset=dup.offset,
                           ap=[dup.ap[0], [2, n], [0, reps // 2], [1, 2]])

        def rsqrt_dve(dst, src, n, eng=None):
            """dst = 1/sqrt(src), quake seed + 1 Newton step, [128, n] f32."""
            e = eng or nc.vector
            qi = small.tile([128, 24], I32, name="qi", tag="qi")[:, :n]
            e.tensor_scalar(out=qi, in0=src.bitcast(I32), scalar1=1,
                            scalar2=None, op0=ALU.logical_shift_right)
            e.tensor_scalar(out=qi, in0=qi, scalar1=-1, scalar2=0x5F3759DF,
                            op0=ALU.mult, op1=ALU.add)
            y0 = qi.bitcast(F32)
            t1 = small.tile([128, 24], F32, name="qt", tag="qt")[:, :n]
            e.tensor_mul(t1, y0, y0)
            e.tensor_mul(t1, t1, src)
            e.tensor_scalar(out=t1, in0=t1, scalar1=-0.5, scalar2=1.5,
                            op0=ALU.mult, op1=ALU.add)
            e.tensor_mul(dst, y0, t1)

        state = {}

        # ---------------- pass A phases ----------------
        def a_dma(t):
            s = state.setdefault(t, {})
            x_t = io.tile([128, DIM], F32, name="x_t")
            nc.sync.dma_start(out=x_t, in_=x_d[t * 128:(t + 1) * 128, :])
            rope_t = io.tile([128, 4 * DHEAD], BF16, name="rope_t")
            nc.sync.dma_start(out=rope_t, in_=rope_d[t * 128:(t + 1) * 128, :])
            s["x"], s["rope"] = x_t, rope_t

        def a_front(t):
            s = state[t]
            if t + 1 < NT:
                a_dma(t + 1)
            x_t = s["x"]

            st = small.tile([128, nc.vector.BN_STATS_DIM], F32, name="st")
            nc.vector.bn_stats(st, x_t)
            # stats = [mean | var | 8 k-sums | 8 q-sums]: bn_aggr's var lands
            # directly at the head of the rsqrt input window
            stats = small.tile([128, 18], F32, name="stats")
            mv = stats[:, 0:2]
            nc.vector.bn_aggr(mv, st)
            s["mv"] = mv
            h = hpool.tile([128, DIM], BF16, name="h_c")
            nc.gpsimd.tensor_scalar(out=h, in0=x_t, scalar1=mv[:, 0:1], scalar2=None,
                                    op0=ALU.subtract)

            pt = psT.tile([128, 512], F32, name="pT", tag="pT")
            for k in range(4):
                nc.tensor.transpose(pt[:, k * 64:k * 64 + 64].bitcast(BF16),
                                    h[:, k * 128:(k + 1) * 128], eye_bf)
            hT = work.tile([128, 4, 128], FP8, name="hT")
            nc.scalar.copy(out=hT, in_=pt[:, 0:256].bitcast(BF16)
                           .rearrange("p (b f) -> p b f", b=4))

            pq = psQ.tile([128, 512], F32, name="pQ", tag="pQ")
            pk = psQ.tile([128, 512], F32, name="pQ", tag="pQ")
            pv = psQ.tile([128, 512], F32, name="pQ", tag="pQ")
            for c, pm in enumerate((pq, pk, pv)):
                for pr in range(2):
                    lhs = _bc(hT[:, 2 * pr, :], [hT.ap[0], [128, 2], [1, 128]])
                    rhs = _bc(wq_sb[:, 2 * pr, c * 512:],
                              [wq_sb.ap[0], [3 * DIM, 2], [1, 512]])
                    nc.tensor.matmul(pm, lhs, rhs, start=(pr == 0), stop=(pr == 1),
                                     perf_mode=DR)
            s["h"] = h
            qsb = work.tile([128, DIM], BF16, name="qsb")
            nc.vector.tensor_copy(qsb, pq)
            ksb = work.tile([128, DIM], BF16, name="ksb")
            nc.vector.tensor_copy(ksb, pk)
            s["qsb"], s["ksb"] = qsb, ksb

            # q/k l2 sums + one 17-wide rsqrt (var | k sums | q sums):
            # unblocks the kT chain and the V scale within this iteration
            k3f = ksb.rearrange("p (h d) -> p h d", h=HEADS)
            q3f = qsb.rearrange("p (h d) -> p h d", h=HEADS)
            sqk = work.tile([128, 2 * DIM], BF16, name="sqk")
            sqk4 = sqk.rearrange("p (h d) -> p h d", h=2 * HEADS)
            ssk = stats[:, 1:18]
            nc.vector.tensor_mul(sqk4[:, 0:HEADS, :], k3f, k3f)
            nc.vector.tensor_mul(sqk4[:, HEADS:, :], q3f, q3f)
            nc.vector.tensor_reduce(out=ssk[:, 1:17], in_=sqk4,
                                    axis=mybir.AxisListType.X, op=ALU.add)
            rnk = small.tile([128, 17], F32, name="rnk17")
            rsqrt_dve(rnk, ssk, 17)
            rnk2 = small.tile([128, 8, 2], BF16, name="rnk2")
            nc.vector.tensor_copy(rnk2, _bc(rnk[:, 1:9], [rnk.ap[0], [1, 8],
                                                          [0, 2]]))
            rn0b = small.tile([128, 1], F32, name="rn0b")
            nc.vector.tensor_scalar(out=rn0b, in0=rnk[:, 0:1],
                                    scalar1=1.0 / W8SCALE, scalar2=None,
                                    op0=ALU.mult)
            rnq8 = small.tile([128, 8], F32, name="rnq8")
            nc.vector.tensor_scalar(out=rnq8, in0=rnk[:, 9:17],
                                    scalar1=QK_SCALE, scalar2=None,
                                    op0=ALU.mult)
            s["rnk2"], s["rnq8"] = rnk2, rnq8

            # V with rstd1 and 1/W8 folded in; pv's PSUM lifetime stays
            # within one iteration
            nc.vector.tensor_scalar(out=vslab[:, t % RING, :], in0=pv,
                                    scalar1=rn0b[:, 0:1], scalar2=None,
                                    op0=ALU.mult)

        def a_mid(t):
            s = state[t]
            qsb, ksb, rope_t = s["qsb"], s["ksb"], s["rope"]
            rnk2 = s.pop("rnk2")
            q3 = qsb.rearrange("p (h d) -> p h d", h=HEADS)
            k3 = ksb.rearrange("p (h d) -> p h d", h=HEADS)

            cos = rope_t[:, 0:DHEAD]
            sin = rope_t[:, DHEAD:2 * DHEAD]
            kcos = rope_t[:, 2 * DHEAD:3 * DHEAD]
            ksin = rope_t[:, 3 * DHEAD:4 * DHEAD]
            cosB = _bc(cos, [cos.ap[0], [0, HEADS], cos.ap[1]])
            sinLo = _bc(sin[:, 0:32], [sin.ap[0], [0, HEADS], [1, 32]])
            sinHi = _bc(sin[:, 32:64], [sin.ap[0], [0, HEADS], [1, 32]])
            kcosB = _bc(kcos, [kcos.ap[0], [0, HEADS], kcos.ap[1]])
            ksinLo = _bc(ksin[:, 0:32], [ksin.ap[0], [0, HEADS], [1, 32]])
            ksinHi = _bc(ksin[:, 32:64], [ksin.ap[0], [0, HEADS], [1, 32]])

            # 1) q path first: rotate-halves (DVE) + combine, then hand the
            # transpose to the DMA xbar immediately (norm folds into exp)
            qs = work.tile([128, DIM], BF16, name="qs")
            qs3 = qs.rearrange("p (h d) -> p h d", h=HEADS)
            qsP = bass.AP(tensor=qs.tensor, offset=qs.offset,
                          ap=[qs.ap[0], [64, 8], [32, 2], [1, 32]])
            qshift = bass.AP(tensor=qsb.tensor, offset=qsb.offset + 32,
                             ap=[qsb.ap[0], [64, 8], [-32, 2], [1, 32]])
            sinP = _bc(sin, [sin.ap[0], [0, HEADS], [32, 2], [1, 32]])
            nc.gpsimd.tensor_mul(qsP, qshift, sinP)
            qr = work.tile([128, DIM], BF16, name="qr")
            qr3 = qr.rearrange("p (h d) -> p h d", h=HEADS)
            nc.vector.tensor_mul(qr3, q3, cosB)
            nc.gpsimd.tensor_add(qr3, qr3, qs3)
            qT4 = work.tile([128, 4, 128], BF16, name="qT4")
            nc.sync.dma_start_transpose(out=qT4[:, :, :], in_=qr)
            s["qT4"] = qT4

            # 2) k path: rotate-halves (Pool), combine + norm scale (DVE,
            # rsqrt already done in a_front), then PE transpose into kslab
            ks_t = work.tile([128, DIM], BF16, name="ks_t")
            ks3 = ks_t.rearrange("p (h d) -> p h d", h=HEADS)
            ksP = bass.AP(tensor=ks_t.tensor, offset=ks_t.offset,
                          ap=[ks_t.ap[0], [64, 8], [32, 2], [1, 32]])
            kshift = bass.AP(tensor=ksb.tensor, offset=ksb.offset + 32,
                             ap=[ksb.ap[0], [64, 8], [-32, 2], [1, 32]])
            ksinP = _bc(ksin, [ksin.ap[0], [0, HEADS], [32, 2], [1, 32]])
            nc.gpsimd.tensor_mul(ksP, kshift, ksinP)
            kr = work.tile([128, DIM], BF16, name="kr")
            kr3 = kr.rearrange("p (h d) -> p h d", h=HEADS)
            nc.vector.tensor_mul(kr3, k3, kcosB)
            nc.gpsimd.tensor_add(kr3, kr3, ks3)
            krP = bass.AP(tensor=kr.tensor, offset=kr.offset,
                          ap=[kr.ap[0], [64, 8], [2, 32], [1, 2]])
            nc.vector.tensor_mul(krP, krP, pair_bc(rnk2, 8, 64))

            ptk = psT.tile([128, 512], F32, name="pT", tag="pT")
            for hd in range(HEADS):
                po = 64 * (hd % 2)
                nc.tensor.transpose(
                    ptk[po:po + 64, (hd // 2) * 64:(hd // 2) * 64 + 64]
                    .bitcast(BF16),
                    kr[:, hd * 64:(hd + 1) * 64], eye_bf)
            nc.scalar.copy(out=kslab[:, t % RING, :, :].bitcast(F32),
                           in_=ptk[:, 0:256].rearrange("p (c f) -> p c f", c=4))

        def a_tSa(t):
            s = state[t]
            rnq8 = s.pop("rnq8")
            qT4 = s.pop("qT4")
            A_sb = apool.tile([128, 4, 512], F16, name="A_sb")
            s["A"] = A_sb
            rsf8 = small.tile([128, 8], F32, name="rsf8", tag="rsf8")
            s["rsf8"] = rsf8
            for g in range(4):
                pb = psS.tile([128, 512], F32, name="pS", tag="pS")
                # causal mask preload on the diagonal blocks (PE accumulate);
                # one pending accumulation group per bank at a time
                for hh in range(2):
                    hd = 2 * g + hh
                    po = 64 * (hd % 2)
                    lhs = qT4[po:po + 64, hd // 2, :]
                    nc.tensor.matmul(pb[:, hh * 256 + WIN:hh * 256 + 2 * WIN],
                                     eye_bf, mask_sb, start=True, stop=False)
                    nc.tensor.matmul(pb[:, hh * 256 + WIN:hh * 256 + 2 * WIN],
                                     lhs,
                                     kslab[po:po + 64, t % RING, hd // 2, :],
                                     start=False, stop=True)
                    if t > 0:
                        nc.tensor.matmul(pb[:, hh * 256:hh * 256 + WIN],
                                         lhs,
                                         kslab[po:po + 64, (t - 1) % RING,
                                               hd // 2, :],
                                         start=True, stop=True)
                for hh in range(2):
                    hd = 2 * g + hh
                    if t > 0:
                        src_ap = pb[:, hh * 256:hh * 256 + 256]
                        dst_ap = A_sb[:, g, hh * 256:hh * 256 + 256]
                    else:
                        src_ap = pb[:, hh * 256 + WIN:hh * 256 + 2 * WIN]
                        dst_ap = A_sb[:, g, hh * 256 + WIN:hh * 256 + 2 * WIN]
                    nc.scalar.activation(out=dst_ap, in_=src_ap, func=AF.Exp,
                                         scale=rnq8[:, hd:hd + 1],
                                         accum_out=rsf8[:, hd:hd + 1])

        def a_tSb(t):
            s = state[t]
            rsf8 = s.pop("rsf8")
            A_sb = s["A"]
            ri8 = small.tile([128, 8], F32, name="ri8", tag="ri8")
            nc.vector.reciprocal(ri8, rsf8)
            rif2 = small.tile([128, 8, 2], F16, name="rif2", tag="rif2")
            nc.vector.tensor_copy(rif2, _bc(ri8[:, :], [ri8.ap[0], [1, 8],
                                                        [0, 2]]))
            # normalize A rows in place (packed-pair APs keep DVE 2x mode)
            if t > 0:
                for G in range(2):
                    sl = A_sb[:, 2 * G, :]
                    slP = bass.AP(tensor=sl.tensor, offset=sl.offset,
                                  ap=[sl.ap[0], [256, 4], [2, 128], [1, 2]])
                    rP = bass.AP(tensor=rif2.tensor,
                                 offset=rif2.offset + 8 * G,
                                 ap=[rif2.ap[0], [2, 4], [0, 128], [1, 2]])
                    eng = nc.vector if G == 0 else nc.gpsimd
                    eng.tensor_mul(slP, slP, rP)
            else:
                for g in range(4):
                    for hh in range(2):
                        sl = A_sb[:, g, hh * 256 + WIN:hh * 256 + 2 * WIN]
                        slP = bass.AP(tensor=sl.tensor, offset=sl.offset,
                                      ap=[sl.ap[0], [2, 64], [1, 2]])
                        rP = bass.AP(tensor=rif2.tensor,
                                     offset=rif2.offset + (2 * g + hh) * 2,
                                     ap=[rif2.ap[0], [0, 64], [1, 2]])
                        nc.vector.tensor_mul(slP, slP, rP)

        def a_back(t):
            s = state.pop(t)
            A_sb, x_t = s["A"], s["x"]
            # A transposes fused with diag(1/rowsum): prev blocks b0, cur b1
            AT = work.tile([128, 2, HEADS, 128], F16, name="AT")
            blks = ([1] if t == 0 else [0, 1])
            for b in blks:
                pa = psT.tile([128, 512], F32, name="pT", tag="pT")
                for hd in range(HEADS):
                    g, hh = hd // 2, hd % 2
                    nc.tensor.transpose(pa[:, hd * 64:hd * 64 + 64].bitcast(F16),
                                        A_sb[:, g, hh * 256 + b * 128:
                                             hh * 256 + b * 128 + 128],
                                        eye_f16)
                if b == 0:
                    nc.vector.tensor_copy(AT[:, b, :, :].bitcast(F32),
                                          pa[:, 0:512].rearrange("p (h f) -> p h f", h=8))
                else:
                    nc.scalar.copy(out=AT[:, b, :, :].bitcast(F32),
                                   in_=pa[:, 0:512].rearrange("p (h f) -> p h f", h=8))

            # AV (two heads stacked per PSUM bank: head h at partitions
            # 64*(h%2), col chunk h//2) + projection
            PTsb = work.tile([128, 4, 128], FP8, name="PTsb")
            pp = psY.tile([128, 512], F32, name="pY", tag="pY")
            for hd in range(HEADS):
                vsl = slice(hd * DHEAD, (hd + 1) * DHEAD)
                po = 64 * (hd % 2)
                out = pp[po:po + 64, (hd // 2) * 128:(hd // 2) * 128 + 128]
                if t == 0:
                    nc.tensor.matmul(out, vslab[:, 0, vsl], AT[:, 1, hd, :],
                                     start=True, stop=True)
                else:
                    nc.tensor.matmul(out, vslab[:, (t - 1) % RING, vsl],
                                     AT[:, 0, hd, :], start=True, stop=False)
                    nc.tensor.matmul(out, vslab[:, t % RING, vsl],
                                     AT[:, 1, hd, :], start=False, stop=True)
            nc.scalar.copy(out=PTsb,
                           in_=pp[:, 0:512].rearrange("p (h f) -> p h f", h=4))

            py = psY.tile([128, 512], F32, name="pY", tag="pY")
            nc.tensor.matmul(py, eyeW8, s["h"], start=True, stop=False)
            for c2 in range(2):
                lhs = _bc(PTsb[:, 2 * c2, :], [PTsb.ap[0], [128, 2], [1, 128]])
                rhs = _bc(wo_sb[:, 2 * c2, :], [wo_sb.ap[0], [DIM, 2], [1, 512]])
                nc.tensor.matmul(py, lhs, rhs, start=False, stop=(c2 == 1),
                                 perf_mode=DR)
            s["py"] = py
            state[t] = s

        def a_x2(t):
            s = state.pop(t)
            nc.scalar.activation(out=x2_slab[:, t, :], in_=s["py"],
                                 func=AF.Identity, scale=1.0 / W8SCALE,
                                 bias=s["mv"][:, 0:1])

        # ---------------- pass B phases ----------------
        bstate = {}

        def b_front(t):
            x2 = x2_slab[:, t, :]
            st2 = small.tile([128, nc.vector.BN_STATS_DIM], F32, name="st2")
            nc.vector.bn_stats(st2, x2)
            mv2 = small.tile([128, nc.vector.BN_AGGR_DIM], F32, name="mv2")
            nc.vector.bn_aggr(mv2, st2)
            rstd2 = small.tile([128, 1], F32, name="rstd2")
            rsqrt_dve(rstd2, mv2[:, 1:2], 1)
            rstd2b = small.tile([128, 1], F32, name="rstd2b")
            nc.vector.tensor_scalar(out=rstd2b, in0=rstd2, scalar1=1.0 / W8SCALE,
                                    scalar2=None, op0=ALU.mult)
            h2 = work.tile([128, DIM], BF16, name="h2")
            nc.vector.tensor_scalar(out=h2, in0=x2,
                                    scalar1=mv2[:, 0:1],
                                    scalar2=rstd2b[:, 0:1],
                                    op0=ALU.subtract, op1=ALU.mult)
            pt = psT.tile([128, 512], F32, name="pT", tag="pT")
            for k in range(4):
                nc.tensor.transpose(pt[:, k * 64:k * 64 + 64].bitcast(BF16),
                                    h2[:, k * 128:(k + 1) * 128], eye_bf)
            h2T = work.tile([128, 4, 128], FP8, name="h2T")
            nc.vector.tensor_copy(h2T, pt[:, 0:256].bitcast(BF16)
                                  .rearrange("p (b f) -> p b f", b=4))
            bstate[t] = h2T

        def b_mm(t):
            h2T = bstate.pop(t)
            gsT = work.tile([128, 16, 128], FP8, name="gsT")
            for bk in range(4):
                pfpool = psQ if bk % 2 == 0 else psS
                pf = pfpool.tile([128, 512], F32, name="pF",
                                 tag="pQ" if bk % 2 == 0 else "pS")
                for c4 in range(4):
                    c = bk * 4 + c4
                    for pr in range(2):
                        lhs = _bc(wf1_sb[:, 2 * pr, c * 128:],
                                  [wf1_sb.ap[0], [4 * DIM, 2], [1, 128]])
                        rhs = _bc(h2T[:, 2 * pr, :], [h2T.ap[0], [128, 2], [1, 128]])
                        nc.tensor.matmul(pf[:, c4 * 128:(c4 + 1) * 128],
                                         lhs, rhs, start=(pr == 0), stop=(pr == 1),
                                         perf_mode=DR)
                nc.scalar.activation(out=gsT[:, 4 * bk:4 * bk + 4, :],
                                     in_=pf.rearrange("p (h f) -> p h f", h=4),
                                     func=AF.Gelu)
            py2 = psY.tile([128, 512], F32, name="pY", tag="pY")
            for j in range(8):
                lhs = _bc(gsT[:, 2 * j, :], [gsT.ap[0], [128, 2], [1, 128]])
                rhs = _bc(wf2_sb[:, 2 * j, :], [wf2_sb.ap[0], [DIM, 2], [1, 512]])
                nc.tensor.matmul(py2, lhs, rhs, start=(j == 0), stop=(j == 7),
                                 perf_mode=DR)
            out_t = work.tile([128, DIM], F32, name="out_t")
            nc.vector.scalar_tensor_tensor(
                out=out_t, in0=py2, scalar=1.0 / W8SCALE,
                in1=x2_slab[:, t, :], op0=ALU.mult, op1=ALU.add)
            nc.sync.dma_start(out=out_d[t * 128:(t + 1) * 128, :], in_=out_t)

        # ---------------- schedule ----------------
        # Pass B is interleaved in chunks of 4 tiles so the exp<->gelu
        # activation-table reload (1.4us) amortizes over 4 tiles.
        def b_chunk(t0, n):
            b_front(t0)
            for tau in range(t0, t0 + n):
                if tau + 1 < t0 + n:
                    b_front(tau + 1)
                b_mm(tau)

        # depth-2 software pipeline: every stage consumes products made a
        # full iteration earlier, so the loop-carried latency chain never
        # paces the steady state
        a_dma(0)
        for i in range(NT + 3):
            if 1 <= i <= 20:
                load_ffn_weights(i - 1)
            if i < NT:
                a_front(i)
            if 1 <= i <= NT:
                a_mid(i - 1)
            if 2 <= i <= NT + 1:
                a_tSa(i - 2)
                a_tSb(i - 2)
            if 3 <= i <= NT + 2:
                a_back(i - 3)
                a_x2(i - 3)
        b_chunk(0, NT)

    nc.compile()
    return nc


_CACHE = {}


def prepare(x, w_qkv, q_scale, k_scale, w_out, b_out, ln1_g, ln1_b,
            ff_ln_g, ff_ln_b, w_ff1, w_ff2):
    x = np.asarray(x, np.float32)
    ln1_g = np.asarray(ln1_g, np.float32)
    ln1_b = np.asarray(ln1_b, np.float32)
    ff_ln_g = np.asarray(ff_ln_g, np.float32)
    ff_ln_b = np.asarray(ff_ln_b, np.float32)
    w_qkv = np.asarray(w_qkv, np.float32)
    w_ff1 = np.asarray(w_ff1, np.float32)
    b_out = np.asarray(b_out, np.float32)
    assert not np.any(w_qkv @ ln1_b), "qkv bias path not supported"
    assert not np.any(w_ff1 @ ff_ln_b), "ff bias path not supported"
    assert not np.any(b_out), "out bias path not supported"

    _bf = mybir.dt.np(BF16)
    _f8 = mybir.dt.np(FP8)
    _f16 = np.float16
    W8 = 8.0
    wqkvT = np.ascontiguousarray((w_qkv * ln1_g[None, :]).T * W8).astype(_f8)
    # rows of w_out.T reordered to the stacked-AV layout: new row c*128+p
    # holds old row (2c + p//64)*64 + p%64
    _c = np.arange(512) // 128
    _p = np.arange(512) % 128
    _idx = (2 * _c + _p // 64) * 64 + _p % 64
    woutT = np.ascontiguousarray((np.asarray(w_out, np.float32).T * W8)[_idx]
                                 ).astype(_f8)
    wff1T = np.ascontiguousarray((w_ff1 * ff_ln_g[None, :]).T * W8).astype(_f8)
    wff2T = np.ascontiguousarray(np.asarray(w_ff2, np.float32).T * W8).astype(_f8)

    pos = np.arange(NTOK, dtype=np.float32)
    inv_freq = 1.0 / (10000.0 ** (np.arange(0, DHEAD, 2, dtype=np.float32) / DHEAD))
    freqs = pos[:, None] * inv_freq
    emb = np.concatenate([freqs, freqs], axis=-1)
    cos, sin = np.cos(emb), np.sin(emb)
    qs = np.asarray(q_scale, np.float32)
    ks = np.asarray(k_scale, np.float32)
    rp = np.concatenate([qs[32:], qs[:32]])
    kp = np.concatenate([ks[32:], ks[:32]])
    sgn = np.concatenate([-np.ones(32, np.float32), np.ones(32, np.float32)])
    qcos = cos * qs[None, :]
    qsin = sin * rp[None, :] * sgn[None, :]
    kcos = cos * ks[None, :]
    ksin = sin * kp[None, :] * sgn[None, :]
    rope = np.concatenate([qcos, qsin, kcos, ksin], axis=1).astype(_bf)

    i_idx = np.arange(WIN)[:, None]
    j_idx = np.arange(WIN)[None, :]
    mask = np.where(i_idx >= j_idx, 0.0, -30000.0).astype(_bf)

    if "nc" not in _CACHE:
        _CACHE["nc"] = build_program()
    nc = _CACHE["nc"]

    shared = dict(wqkvT=wqkvT, woutT=woutT, wff1T=wff1T, wff2T=wff2T,
                  rope=rope, mask=mask)
    in_maps = [dict(x=np.ascontiguousarray(x[i]), **shared) for i in range(B)]
    return nc, in_maps


def kernel(x, w_qkv, q_scale, k_scale, w_out, b_out, ln1_g, ln1_b,
           ff_ln_g, ff_ln_b, w_ff1, w_ff2, **run_kwargs):
    nc, in_maps = prepare(x, w_qkv, q_scale, k_scale, w_out, b_out, ln1_g,
                          ln1_b, ff_ln_g, ff_ln_b, w_ff1, w_ff2)
    res = run_bass_kernel_spmd(nc, in_maps, list(range(B)), **run_kwargs)
    out = np.stack([res.results[i]["out"] for i in range(B)]).astype(np.float32)
    if run_kwargs:
        return out, res
    return out

